# revision 6
# baseline (speedup 1.0000x reference)
"""Trainium2 Bass kernel for KVCache.update.

Semantics (matching the reference):
  - scatter xk/xv into k_cache/v_cache at [layer_idx, :, curr_pos:curr_pos+S]
    producing fresh full-cache outputs k_new/v_new
  - gather the prefix [:curr_pos+S] of the updated layer and
    repeat_interleave KV heads by n_rep (GQA) producing keys/values

Sharding: tensor-parallel over the 8 KV heads (1 head per NeuronCore).
Each core:
  - copies its (LAYERS, BSZ, MAX_SEQ, HD) cache shard HBM->HBM with two
    seq_len*HD holes left at the scatter target, then DMAs xk/xv into the
    holes (all writes disjoint -> no ordering hazards)
  - gathers the layer_idx prefix once into SBUF (partition = 16-row chunk
    so the store is fully contiguous in DRAM), replicates x n_rep with
    on-chip copies, and stores keys/values without re-reading HBM n_rep
    times.
"""

import numpy as np
import ml_dtypes

N_CORES = 8

_BUILD_CACHE = {}


def _build_bass(layers, bsz, max_seq, hd, seq_len, layer_idx, curr_pos, n_rep):
    import concourse.bass as bass
    import concourse.mybir as mybir

    dt = mybir.dt.bfloat16
    total = curr_pos + seq_len
    nc = bass.Bass()

    kc = nc.declare_dram_parameter("kc", [layers, bsz, max_seq, hd], dt, isOutput=False)
    vc = nc.declare_dram_parameter("vc", [layers, bsz, max_seq, hd], dt, isOutput=False)
    xk = nc.declare_dram_parameter("xk", [bsz, seq_len, hd], dt, isOutput=False)
    xv = nc.declare_dram_parameter("xv", [bsz, seq_len, hd], dt, isOutput=False)
    k_new = nc.declare_dram_parameter(
        "k_new", [layers, bsz, max_seq, hd], dt, isOutput=True
    )
    v_new = nc.declare_dram_parameter(
        "v_new", [layers, bsz, max_seq, hd], dt, isOutput=True
    )
    keys = nc.declare_dram_parameter("keys", [bsz, total, n_rep, hd], dt, isOutput=True)
    values = nc.declare_dram_parameter(
        "values", [bsz, total, n_rep, hd], dt, isOutput=True
    )

    assert curr_pos % 128 == 0, "gather layout assumes curr_pos % 128 == 0"
    rpp = curr_pos // 128  # prefix rows per SBUF partition

    def emit(eng, cache, x, new, out, sem, lsem, rsem, srcs, reps):
        """DMA program for one cache on one HWDGE ring."""
        n = 0
        # gather prefix loads into SBUF: partition q holds rows 16q..16q+15
        # so the keys store below is DRAM-contiguous per partition.
        for b in range(bsz):
            eng.dma_start(
                srcs[b][:],
                cache[layer_idx, b, 0:curr_pos, :].rearrange(
                    "(q i) d -> q (i d)", q=128
                ),
            ).then_inc(lsem, 16)
        # full-shard HBM->HBM copy, skipping the scatter holes.  Spans are
        # ordered so ~12MB is queued ahead of the rsem wait below.
        flat_src = cache[:].rearrange("a b c d -> (a b c d)")
        flat_dst = new[:].rearrange("a b c d -> (a b c d)")
        n_elems = layers * bsz * max_seq * hd
        hole_len = seq_len * hd
        spans = []
        pos = 0
        for b in range(bsz):
            h = ((layer_idx * bsz + b) * max_seq + curr_pos) * hd
            if h > pos:
                spans.append((pos, h))
            pos = h + hole_len
        if n_elems > pos:
            # split the final (largest) span for finer ring interleaving
            mid = ((pos + n_elems) // 2) // 2048 * 2048
            spans.append((pos, mid))
            spans.append((mid, n_elems))
        spans.sort(key=lambda s: s[0] - s[1])  # largest first

        def span(i):
            lo, hi = spans[i]
            nonlocal n
            eng.dma_start(flat_dst[lo:hi], flat_src[lo:hi]).then_inc(sem, 16)
            n += 1

        span(2)  # ~11MB queued ahead of the stores' sem wait
        # gather tails (seq_len rows): small, via broadcast source
        for b in range(bsz):
            tail = x[b].unsqueeze(1).broadcast_to((seq_len, n_rep, hd))
            eng.dma_start(out[b, curr_pos:total, :, :], tail).then_inc(sem, 16)
            n += 1
        # scatter the new tokens into the cache holes
        eng.dma_start(new[layer_idx, :, curr_pos:total, :], x[:]).then_inc(sem, 16)
        n += 1
        for i in range(3, len(spans)):
            span(i)
        # gather prefix stores (wait for the DVE replicate, then enqueue)
        eng.wait_ge(rsem, 1)
        for b in range(bsz):
            eng.dma_start(
                out[b, 0:curr_pos, :, :].rearrange("(q i) r d -> q (i r d)", q=128),
                reps[b][:],
            ).then_inc(sem, 16)
            n += 1
        span(0)
        span(1)
        eng.wait_ge(sem, 16 * n)

    import contextlib

    with contextlib.ExitStack() as ctx:
        sk = [
            ctx.enter_context(nc.sbuf_tensor(f"sk{b}", [128, rpp * hd], dt))
            for b in range(bsz)
        ]
        sv = [
            ctx.enter_context(nc.sbuf_tensor(f"sv{b}", [128, rpp * hd], dt))
            for b in range(bsz)
        ]
        rk = [
            ctx.enter_context(nc.sbuf_tensor(f"rk{b}", [128, rpp * n_rep * hd], dt))
            for b in range(bsz)
        ]
        rv = [
            ctx.enter_context(nc.sbuf_tensor(f"rv{b}", [128, rpp * n_rep * hd], dt))
            for b in range(bsz)
        ]
        sem_k = ctx.enter_context(nc.semaphore("sem_k"))
        sem_v = ctx.enter_context(nc.semaphore("sem_v"))
        lsem_k = ctx.enter_context(nc.semaphore("lsem_k"))
        lsem_v = ctx.enter_context(nc.semaphore("lsem_v"))
        rsem_k = ctx.enter_context(nc.semaphore("rsem_k"))
        rsem_v = ctx.enter_context(nc.semaphore("rsem_v"))
        block = ctx.enter_context(nc.Block())

        @block.sync
        def _(sync):
            emit(sync, kc, xk, k_new, keys, sem_k, lsem_k, rsem_k, sk, rk)

        @block.scalar
        def _(scalar):
            emit(scalar, vc, xv, v_new, values, sem_v, lsem_v, rsem_v, sv, rv)

        @block.vector
        def _(vector):
            for lsem, rsem, srcs, reps in (
                (lsem_k, rsem_k, sk, rk),
                (lsem_v, rsem_v, sv, rv),
            ):
                vector.wait_ge(lsem, 16 * bsz)
                last = None
                for b in range(bsz):
                    rep4 = reps[b][:].rearrange(
                        "q (i r d) -> q i r d", i=rpp, r=n_rep, d=hd
                    )
                    src3 = srcs[b][:].rearrange("q (i d) -> q i d", i=rpp, d=hd)
                    for r in range(n_rep):
                        last = nc.vector.tensor_copy(rep4[:, :, r, :], src3)
                assert last is not None
                last.then_inc(rsem, 1)

    return nc


def _get_bass(key):
    if key not in _BUILD_CACHE:
        _BUILD_CACHE[key] = _build_bass(*key)
    return _BUILD_CACHE[key]


def _as_bf16(a):
    a = np.asarray(a)
    if a.dtype != ml_dtypes.bfloat16:
        a = a.astype(ml_dtypes.bfloat16)
    return a


def _run(k_cache, v_cache, xk, xv, layer_idx, curr_pos, n_rep, **spmd_kwargs):
    layer_idx = int(layer_idx)
    curr_pos = int(curr_pos)
    n_rep = int(n_rep)

    k_cache = _as_bf16(k_cache)
    v_cache = _as_bf16(v_cache)
    xk = _as_bf16(xk)
    xv = _as_bf16(xv)

    layers, bsz, max_seq, n_kv, hd = k_cache.shape
    seq_len = xk.shape[1]
    assert n_kv == N_CORES, f"expected {N_CORES} KV heads, got {n_kv}"

    nc = _get_bass((layers, bsz, max_seq, hd, seq_len, layer_idx, curr_pos, n_rep))

    in_maps = []
    for c in range(N_CORES):
        in_maps.append(
            {
                "kc": np.ascontiguousarray(k_cache[:, :, :, c, :]),
                "vc": np.ascontiguousarray(v_cache[:, :, :, c, :]),
                "xk": np.ascontiguousarray(xk[:, :, c, :]),
                "xv": np.ascontiguousarray(xv[:, :, c, :]),
            }
        )

    from concourse.bass_utils import run_bass_kernel_spmd

    res = run_bass_kernel_spmd(nc, in_maps, core_ids=list(range(N_CORES)), **spmd_kwargs)
    rs = res.results

    k_new = np.stack([r["k_new"] for r in rs], axis=3)
    v_new = np.stack([r["v_new"] for r in rs], axis=3)
    keys = np.concatenate([r["keys"] for r in rs], axis=2)
    values = np.concatenate([r["values"] for r in rs], axis=2)
    return (keys, values, k_new, v_new), res


def kernel(k_cache, v_cache, xk, xv, layer_idx, curr_pos, n_rep):
    outs, _ = _run(k_cache, v_cache, xk, xv, layer_idx, curr_pos, n_rep)
    return outs
